# revision 37
# baseline (speedup 1.0000x reference)
"""Trainium2 Bass kernel for nn_DatastoreReaderLayer (retrieval kNN attention).

Strategy (8 NeuronCores, datastore sharded over N):
  - Each core owns an N/8 = 4096-row shard of the datastore.
  - K/V weight projections are algebraically absorbed:
      logits = qk @ dstore_k.T   where qk := alpha * (qb @ Wq.T + bq) @ Wk
      attn   = (softmax @ dstore_v) @ Wv.T
    qk (a [1024, 512] projection, 0.25% of total FLOPs) is folded on host;
    the O(N) retrieval work runs on device.
  - fp8 (e4m3) main loop with DoubleRow matmuls (2 k-subtiles per pass,
    0.5 PE cycles/row): logits and AV both contract 256-deep per matmul.
    qk is scaled by QS=64 into fp8 range (undone by the exp's scale);
    exp output is biased by ln(1/2) so e stays within e4m3's +/-240
    (the 1/2 is a global softmax constant and cancels).
  - Softmax without max-subtraction (logits are in [-5.7, 5.5] for this
    distribution; exp fits fp8 with the 1/2 bias).
  - Per-core partial sum-exp (from retained fp8 e tiles, contracted with a
    ones vector per half) and partial unnormalized AV are combined across
    cores with ONE bf16 ReduceScatter over all 1024 query rows ([1024, 513]
    payload: 512 attn columns + 1 sum-exp column).
  - Each core finishes the gate MLP (bf16 weights) for its own 128 query
    rows. Using relu(r*x) = r*relu(x) for r = 1/sum-exp > 0, the prev-side
    gate matmul + fused bias row run during the ReduceScatter; only the
    attn-side matmul and a short fused elementwise chain remain after it.
  - bk is provably a softmax no-op; bv/bg1/bg2 are applied exactly on device.
"""

import sys

for _p in ("/opt/trn_rl_repo", "/root/.axon_site/_ro/trn_rl_repo"):
    if _p not in sys.path:
        sys.path.append(_p)

import numpy as np
import ml_dtypes

import concourse.tile as tile
from concourse import bacc, mybir
from concourse.bass_utils import run_bass_kernel_spmd

SEQ, BATCH, D, NTOT = 256, 4, 512, 32768
TEMP = 0.5
NCORES = 8
SB = SEQ * BATCH  # 1024 query rows, b-major (row r = b*SEQ + s)
F32 = mybir.dt.float32
BF16 = mybir.dt.bfloat16
F8 = mybir.dt.float8e4
AF = mybir.ActivationFunctionType
ALU = mybir.AluOpType
FP8_NP = ml_dtypes.float8_e4m3
BF16_NP = ml_dtypes.bfloat16

QS = 64.0            # qk pre-scale into fp8 normal range
EBIAS = -0.6931472   # exp bias: e' = 0.5 * e^l keeps e' <= 120 < 240 (fp8 max)

_PROGRAM_CACHE: dict = {}


def build_program(ns: int, bg2f: float, reps: int = 1):
    """One SPMD program; per-core data differences come via in_maps.

    reps > 1 statically repeats the whole computation (for wall-clock-delta
    timing of the kernel proper); the output is written identically each rep.
    """
    nchunks = ns // 128          # 32
    npairs = nchunks // 2        # 16 chunk-pairs (DoubleRow contracts 2)
    nc = bacc.Bacc(None, target_bir_lowering=False, debug=False, num_devices=NCORES)

    def inp(nm, shp, dt=F32):
        return nc.declare_dram_parameter(nm, list(shp), dt, isOutput=False)

    qkT8_d = inp("qkT8", (128, 4, SB), F8)       # [p, k, s]: qk[s, k*128+p]*QS
    dkT8_d = inp("dkT8", (128, 4, ns), F8)       # [p, k, n]: dk[n, k*128+p]
    dv8_d = inp("dv8", (128, npairs, 2, D), F8)  # [p, jp, c, d]: dv[jp*256+c*128+p, d]
    ones8_d = inp("ones8", (128, 2, 128), F8)
    wvT_d = inp("wvT", (D, D))            # Wv.T
    wg1T_d = inp("wg1T", (2 * D, D), BF16)  # Wg1.T
    wg2r_d = inp("wg2r", (128, D))        # Wg2 replicated over partitions
    bvr_d = inp("bvr", (128, D))
    bvg1_d = inp("bvg1", (1, D), BF16)    # bv @ Wg1a.T + bg1 (gate bias row)
    prevN_d = inp("prevN", (128, D))      # prev rows for this core's slice
    prevT_d = inp("prevT", (D, 128), BF16)  # same rows, transposed
    ident_d = inp("ident", (128, 128), BF16)
    out_d = nc.declare_dram_parameter("out", [128, D], F32, isOutput=True)

    rg = [list(range(NCORES))]
    DR = mybir.MatmulPerfMode.DoubleRow

    def emit_body(nc, tc, pools, rp):
        cp, sp, ep, xp, wgp, mmp, wvp, dp = pools

        def r32(ap):
            return ap.bitcast(mybir.dt.float32r)

        def cload(src_ap, shape, tg, dt=F32, eng=None, rounded=False):
            t = cp.tile(shape, dt, tag=tg, name=rp + tg)
            if rounded:
                (eng or nc.sync).dma_start(r32(t[:]), r32(src_ap))
            else:
                (eng or nc.sync).dma_start(t[:], src_ap)
            return t

        # --- loads ---------------------------------------------------------
        # SP(sync) queue: qk + datastore-K streams; ACT queue: datastore-V
        # (parallel DMA rings; ACT's issues land before the first exp).
        qkT8 = cp.tile([128, 4, SB], F8, tag="qkT8", name=rp + "qkT8")
        nc.sync.dma_start(qkT8[:, :, 0:512], qkT8_d[:, :, 0:512])
        dkT8 = cp.tile([128, 4, ns], F8, tag="dkT8", name=rp + "dkT8")
        nc.sync.dma_start(dkT8[:, :, 0:512], dkT8_d[:, :, 0:512])
        dv8 = cp.tile([128, npairs, 2, D], F8, tag="dv8", name=rp + "dv8")
        nc.sync.dma_start(dv8[:, 0:4, :, :], dv8_d[:, 0:4, :, :])
        nc.sync.dma_start(dkT8[:, :, 512:2048], dkT8_d[:, :, 512:2048])
        nc.sync.dma_start(dv8[:, 4:10, :, :], dv8_d[:, 4:10, :, :])
        nc.sync.dma_start(dkT8[:, :, 2048:ns], dkT8_d[:, :, 2048:ns])
        nc.sync.dma_start(dv8[:, 10:npairs, :, :], dv8_d[:, 10:npairs, :, :])
        nc.sync.dma_start(qkT8[:, :, 512:SB], qkT8_d[:, :, 512:SB])
        # remaining constants on the SP queue, behind the main streams
        ones8 = cload(ones8_d[:], [128, 2, 128], "ones8", dt=F8)
        wvT = [cload(wvT_d[k * 128:(k + 1) * 128, :], [128, D], f"wvT{k}",
                     rounded=True) for k in range(4)]
        wg2r = cload(wg2r_d[:], [128, D], "wg2r")
        bvr = cload(bvr_d[:], [128, D], "bvr")
        bvg1 = cload(bvg1_d[:], [1, D], "bvg1", dt=BF16)
        prevN = cload(prevN_d[:], [128, D], "prevN")
        prevT = [cload(prevT_d[k * 128:(k + 1) * 128, :], [128, 128], f"prevT{k}",
                       dt=BF16) for k in range(4)]
        ident = cload(ident_d[:], [128, 128], "identb", dt=BF16)
        ones = cp.tile([128, 1], F32, tag="ones", name=rp + "ones")
        nc.vector.memset(ones[:], 1.0)
        ebias = cp.tile([128, 1], F32, tag="ebias", name=rp + "ebias")
        nc.vector.memset(ebias[:], EBIAS)
        # [1,128] bf16 ones row: rank-1 bias accumulation in the gate matmul
        onesr = cp.tile([1, 128], BF16, tag="onesr", name=rp + "onesr")
        nc.vector.memset(onesr[:], 1.0)

        wvacc = [cp.tile([128, SB], F32, tag=f"wvacc{k}", name=rp + f"wvacc{k}")
                 for k in range(4)]
        S_sb = cp.tile([1, SB], F32, tag="S_sb", name=rp + "S_sb")
        eT = {}  # (h, jp) -> retained fp8 e tile

        cc_in = dp.tile([SB, 513], BF16, tag="ccin", name=rp + "ccin")
        cc_out = dp.tile([SB // NCORES, 513], BF16, tag="ccout", name=rp + "ccout")

        # --- main loop: logits -> exp -> AV, fp8 DoubleRow -----------------
        for h in range(2):
            sc = slice(h * 512, (h + 1) * 512)
            wv_ps = [wvp.tile([128, 512], F32, tag="wv", name=rp + f"wv{h}{k}")
                     for k in range(4)]
            for jp in range(npairs):
                pl2 = mmp.tile([128, 1024], F32, tag="mm", name=rp + f"pl{h}{jp}")
                for c in range(2):
                    j = jp * 2 + c
                    for kp in range(2):
                        nc.tensor.matmul(
                            pl2[:, c * 512:(c + 1) * 512],
                            dkT8[:, 2 * kp:2 * kp + 2, j * 128:(j + 1) * 128],
                            qkT8[:, 2 * kp:2 * kp + 2, sc],
                            start=(kp == 0), stop=(kp == 1), perf_mode=DR)
                et = ep.tile([128, 1024], F8, tag="e", name=rp + f"e{h}{jp}")
                nc.scalar.activation(et[:], pl2[:], AF.Exp,
                                     scale=1.0 / QS, bias=ebias[:])
                eT[(h, jp)] = et
                e3 = et[:].rearrange("p (c q) -> p c q", c=2)
                for k in range(4):
                    nc.tensor.matmul(
                        wv_ps[k][:], dv8[:, jp, :, k * 128:(k + 1) * 128], e3,
                        start=(jp == 0), stop=(jp == npairs - 1), perf_mode=DR)
            for k in range(4):
                nc.vector.tensor_copy(r32(wvacc[k][:, sc]), wv_ps[k][:])
            # sum-exp over this half's retained e tiles; for h=0 these PE ops
            # slot in at the head of h=1's loop, off the critical path.
            se_ps = wvp.tile([128, 512], F32, tag="wv", name=rp + f"se{h}")
            for jp in range(npairs):
                nc.tensor.matmul(
                    se_ps[:], ones8[:],
                    eT[(h, jp)][:].rearrange("p (c q) -> p c q", c=2),
                    start=(jp == 0), stop=(jp == npairs - 1), perf_mode=DR)
            nc.vector.tensor_copy(S_sb[0:1, sc], se_ps[0:1, :])

        # --- per-128-query-group: project partial AV through Wv.T, ship ----
        for g in range(8):
            pa = mmp.tile([128, 512], F32, tag="mm", name=rp + f"pa{g}")
            for k in range(4):
                nc.tensor.matmul(
                    pa[:], r32(wvacc[k][:, g * 128:(g + 1) * 128]),
                    r32(wvT[k][:]), start=(k == 0), stop=(k == 3))
            psc = wvp.tile([128, 1], F32, tag="wv", name=rp + f"psc{g}")
            nc.tensor.matmul(psc[:], S_sb[0:1, g * 128:(g + 1) * 128],
                             ones[0:1, 0:1], start=True, stop=True)
            ext = xp.tile([128, 513], BF16, tag="ext", name=rp + f"ext{g}")
            # ACT is idle after the exp stream: do the wide PSUM->SBUF copies
            # there; DVE handles the 1-column sum-exp copies.
            nc.scalar.activation(ext[:, 0:512], pa[:], AF.Identity)
            nc.vector.tensor_copy(ext[:, 512:513], psc[:])
            nc.sync.dma_start(cc_in[g * 128:(g + 1) * 128, :], ext[:])
        # wg1T streamed now (bf16): the transfers ride out the RS wait.
        wg1T = []
        for k in range(8):
            t = wgp.tile([128, D], BF16, tag="wg", name=rp + f"wg1T{k}")
            nc.sync.dma_start(t[:], wg1T_d[k * 128:(k + 1) * 128, :])
            wg1T.append(t)
        # Gate algebra: with A := raw reduced AV (pre-recip), r := 1/S,
        #   h = relu(cat[A*r + bv, prev] @ Wg1.T + bg1)
        #     = r * relu(A @ Wg1a.T + S*(prev @ Wg1b.T + bv@Wg1a.T + bg1))
        # so the prev-side matmul + bias run DURING the RS, and r folds into
        # the sigmoid input scalar afterwards.
        phB = mmp.tile([128, D], F32, tag="mm", name=rp + "phB")
        for k in range(4):
            nc.tensor.matmul(phB[:], prevT[k][:], wg1T[4 + k][:],
                             start=(k == 0), stop=False)
        nc.tensor.matmul(phB[:], onesr[:], bvg1[:], start=False, stop=True)
        phBs = cp.tile([128, D], F32, tag="phBs", name=rp + "phBs")
        nc.vector.tensor_copy(phBs[:], phB[:])
        nc.gpsimd.collective_compute(
            "ReduceScatter", ALU.add, replica_groups=rg,
            ins=[cc_in.opt()], outs=[cc_out.opt()])

        # --- post-RS: this core's 128 query rows ---------------------------
        post = cp.tile([128, 513], BF16, tag="post", name=rp + "post")
        nc.sync.dma_start(post[:], cc_out[:])
        recip = cp.tile([128, 1], F32, tag="recip", name=rp + "recip")
        nc.vector.reciprocal(recip[:], post[:, 512:513])
        attn = sp.tile([128, D], F32, tag="scr", name=rp + "attn")
        nc.vector.scalar_tensor_tensor(
            attn[:], post[:, 0:512], recip[:], bvr[:],
            op0=ALU.mult, op1=ALU.add)
        dlt = sp.tile([128, D], F32, tag="scr", name=rp + "dlt")
        nc.vector.tensor_tensor(dlt[:], attn[:], prevN[:], op=ALU.subtract)

        aTall = cp.tile([128, D], BF16, tag="aTall", name=rp + "aTall")
        for k in range(4):
            pt = wvp.tile([128, 128], BF16, tag="wv", name=rp + f"pt{k}")
            nc.tensor.transpose(pt[:], post[:, k * 128:(k + 1) * 128], ident[:])
            nc.vector.tensor_copy(aTall[:, k * 128:(k + 1) * 128], pt[:])
        phA = mmp.tile([128, D], F32, tag="mm", name=rp + "phA")
        for k in range(4):
            nc.tensor.matmul(phA[:], aTall[:, k * 128:(k + 1) * 128],
                             wg1T[k][:], start=(k == 0), stop=(k == 3))
        x = sp.tile([128, D], F32, tag="scr", name=rp + "x")
        nc.vector.scalar_tensor_tensor(
            x[:], phBs[:], post[:, 512:513], phA[:], op0=ALU.mult, op1=ALU.add)
        hrelu = sp.tile([128, D], F32, tag="scr", name=rp + "hrelu")
        nc.scalar.activation(hrelu[:], x[:], AF.Relu)

        tmp = sp.tile([128, D], F32, tag="scr", name=rp + "tmp")
        sigp = cp.tile([128, 1], F32, tag="sigp", name=rp + "sigp")
        nc.vector.scalar_tensor_tensor(
            tmp[:], hrelu[:], 1.0, wg2r[:],
            op0=ALU.mult, op1=ALU.mult, accum_out=sigp[:])
        sigin = cp.tile([128, 1], F32, tag="sigin", name=rp + "sigin")
        nc.vector.tensor_tensor(sigin[:], sigp[:], recip[:], op=ALU.mult)
        # sigma = 0.5 + 0.5*tanh(0.5*(x + bg2)); tanh shares the Exp table set
        tnh = cp.tile([128, 1], F32, tag="tnh", name=rp + "tnh")
        nc.scalar.activation(tnh[:], sigin[:], AF.Tanh,
                             scale=0.5, bias=0.5 * bg2f)
        sig1 = cp.tile([128, 1], F32, tag="sig1", name=rp + "sig1")
        nc.vector.tensor_scalar(sig1[:], tnh[:], 0.5, 0.5,
                                op0=ALU.mult, op1=ALU.add)
        res = sp.tile([128, D], F32, tag="scr", name=rp + "res")
        nc.vector.scalar_tensor_tensor(
            res[:], dlt[:], sig1[:], prevN[:], op0=ALU.mult, op1=ALU.add)
        nc.sync.dma_start(out_d[:], res[:])

    with tile.TileContext(nc) as tc:
        with (
            tc.tile_pool(name="const", bufs=1) as cp,
            tc.tile_pool(name="scratch", bufs=8) as sp,
            tc.tile_pool(name="ep", bufs=32) as ep,
            tc.tile_pool(name="xp", bufs=3) as xp,
            tc.tile_pool(name="wgp", bufs=8) as wgp,
            tc.tile_pool(name="mm", bufs=2, space="PSUM") as mmp,
            tc.tile_pool(name="wvp", bufs=4, space="PSUM") as wvp,
            tc.tile_pool(name="dram", bufs=1, space="DRAM") as dp,
        ):
            pools = (cp, sp, ep, xp, wgp, mmp, wvp, dp)
            for rep in range(reps):
                emit_body(nc, tc, pools, f"r{rep}_" if reps > 1 else "")

    nc.finalize()
    return nc


def make_in_maps(q, prev, Wq, bq, Wk, Wv, Wg1, Wg2, bg2, bv, bg1,
                 dstore_k, dstore_v, ns):
    """Host-side sharding + layout prep. Returns per-core input dicts."""
    alpha = (D ** -0.5) / TEMP
    f = np.float32
    qb = np.ascontiguousarray(q.transpose(1, 0, 2).reshape(SB, D), dtype=f)
    prevb = np.ascontiguousarray(prev.transpose(1, 0, 2).reshape(SB, D), dtype=f)
    wqk = (Wq.T.astype(np.float64) @ Wk.astype(np.float64) * alpha).astype(f)
    qkb = ((bq.astype(np.float64) @ Wk.astype(np.float64)) * alpha).astype(f)
    qk = qb @ wqk + qkb                      # [SB, D] projected scaled queries
    # [p, k, s] fp8 layout, pre-scaled by QS
    qkT8 = np.ascontiguousarray(
        (qk.T * QS).reshape(4, 128, SB).transpose(1, 0, 2)).astype(FP8_NP)
    wvT = np.ascontiguousarray(Wv.T.astype(f))
    wg1T = np.ascontiguousarray(Wg1.T.astype(BF16_NP))
    wg2r = np.ascontiguousarray(np.broadcast_to(Wg2.reshape(1, D), (128, D)), dtype=f)
    bvr = np.ascontiguousarray(np.broadcast_to(bv.reshape(1, D), (128, D)), dtype=f)
    bvg1 = np.ascontiguousarray(
        (bv.astype(np.float64) @ Wg1.T.astype(np.float64)[0:D]
         + bg1.astype(np.float64)).reshape(1, D)).astype(BF16_NP)
    ident = np.eye(128, dtype=BF16_NP)
    ones8 = np.ones((128, 2, 128), dtype=FP8_NP)
    npairs = ns // 256

    in_maps = []
    for c in range(NCORES):
        rows = slice(c * 128, (c + 1) * 128)
        prevN = np.ascontiguousarray(prevb[rows])
        prevT = np.ascontiguousarray(prevN.T.astype(BF16_NP))
        dk_s = dstore_k[c * ns:(c + 1) * ns, :].astype(f)
        dv_s = dstore_v[c * ns:(c + 1) * ns, :].astype(f)
        dkT8 = np.ascontiguousarray(
            dk_s.T.reshape(4, 128, ns).transpose(1, 0, 2)).astype(FP8_NP)
        dv8 = np.ascontiguousarray(
            dv_s.reshape(npairs, 2, 128, D).transpose(2, 0, 1, 3)).astype(FP8_NP)
        in_maps.append({
            "qkT8": qkT8, "dkT8": dkT8, "dv8": dv8, "ones8": ones8,
            "wvT": wvT, "wg1T": wg1T, "wg2r": wg2r, "bvr": bvr, "bvg1": bvg1,
            "prevN": prevN, "prevT": prevT, "ident": ident,
        })
    return in_maps


def assemble_output(core_outs):
    """[128,512] per core -> [SEQ, BATCH, D] full output."""
    res_bm = np.empty((SB, D), dtype=np.float32)
    for c in range(NCORES):
        res_bm[c * 128:(c + 1) * 128] = core_outs[c]
    return np.ascontiguousarray(
        res_bm.reshape(BATCH, SEQ, D).transpose(1, 0, 2))


def kernel(q, prev_layer_output, Wq, bq, Wk, bk, Wv, bv, Wg1, bg1, Wg2, bg2,
           dstore_k, dstore_v):
    # bk shifts every logit in a row by a constant -> softmax-invariant; unused.
    ns = NTOT // NCORES
    bg2f = float(np.asarray(bg2).reshape(-1)[0])
    key = (ns, bg2f, 1)
    if key not in _PROGRAM_CACHE:
        _PROGRAM_CACHE[key] = build_program(ns, bg2f)
    nc = _PROGRAM_CACHE[key]
    in_maps = make_in_maps(q, prev_layer_output, Wq, bq, Wk, Wv, Wg1, Wg2, bg2,
                           bv, bg1, dstore_k, dstore_v, ns)
    res = run_bass_kernel_spmd(nc, in_maps, list(range(NCORES)))
    return assemble_output([res.results[c]["out"] for c in range(NCORES)])


# revision 41
# speedup vs baseline: 2.0105x; 2.0105x over previous
"""Trainium2 Bass kernel for nn_DatastoreReaderLayer (retrieval kNN attention).

Strategy (8 NeuronCores, datastore sharded over N):
  - Each core owns an N/8 = 4096-row shard of the datastore.
  - K/V weight projections are algebraically absorbed:
      logits = qk @ dstore_k.T   where qk := alpha * (qb @ Wq.T + bq) @ Wk
      attn   = (softmax @ dstore_v) @ Wv.T
    qk (a [1024, 512] projection, 0.25% of total FLOPs) is folded on host;
    the O(N) retrieval work runs on device.
  - fp8 (e4m3) main loop with DoubleRow matmuls (2 k-subtiles per pass,
    0.5 PE cycles/row): logits and AV both contract 256-deep per matmul.
    qk is scaled by QS=64 into fp8 range (undone by the exp's scale);
    exp output is biased by ln(1/2) so e stays within e4m3's +/-240
    (the 1/2 is a global softmax constant and cancels).
  - Softmax without max-subtraction (logits are in [-5.7, 5.5] for this
    distribution; exp fits fp8 with the 1/2 bias).
  - Per-core partial sum-exp (from retained fp8 e tiles, contracted with a
    ones vector per half) and partial unnormalized AV are combined across
    cores with ONE bf16 ReduceScatter over all 1024 query rows ([1024, 513]
    payload: 512 attn columns + 1 sum-exp column).
  - Each core finishes the gate MLP (bf16 weights) for its own 128 query
    rows. Using relu(r*x) = r*relu(x) for r = 1/sum-exp > 0, the prev-side
    gate matmul + fused bias row run during the ReduceScatter; only the
    attn-side matmul and a short fused elementwise chain remain after it.
  - bk is provably a softmax no-op; bv/bg1/bg2 are applied exactly on device.
"""

import sys

for _p in ("/opt/trn_rl_repo", "/root/.axon_site/_ro/trn_rl_repo"):
    if _p not in sys.path:
        sys.path.append(_p)

import numpy as np
import ml_dtypes

import concourse.tile as tile
from concourse import bacc, mybir
from concourse.bass_utils import run_bass_kernel_spmd

SEQ, BATCH, D, NTOT = 256, 4, 512, 32768
TEMP = 0.5
NCORES = 8
SB = SEQ * BATCH  # 1024 query rows, b-major (row r = b*SEQ + s)
F32 = mybir.dt.float32
BF16 = mybir.dt.bfloat16
F8 = mybir.dt.float8e4
AF = mybir.ActivationFunctionType
ALU = mybir.AluOpType
FP8_NP = ml_dtypes.float8_e4m3
BF16_NP = ml_dtypes.bfloat16

QS = 64.0            # qk pre-scale into fp8 normal range
EBIAS = -0.6931472   # exp bias: e' = 0.5 * e^l keeps e' <= 120 < 240 (fp8 max)

_PROGRAM_CACHE: dict = {}


def build_program(ns: int, bg2f: float, reps: int = 1):
    """One SPMD program; per-core data differences come via in_maps.

    reps > 1 statically repeats the whole computation (for wall-clock-delta
    timing of the kernel proper); the output is written identically each rep.
    """
    nchunks = ns // 128          # 32
    npairs = nchunks // 2        # 16 chunk-pairs (DoubleRow contracts 2)
    nc = bacc.Bacc(None, target_bir_lowering=False, debug=False, num_devices=NCORES)

    def inp(nm, shp, dt=F32):
        return nc.declare_dram_parameter(nm, list(shp), dt, isOutput=False)

    qkT8_d = inp("qkT8", (128, 4, SB), F8)       # [p, k, s]: qk[s, k*128+p]*QS
    dkT8_d = inp("dkT8", (128, 4, ns), F8)       # [p, k, n]: dk[n, k*128+p]
    dv8_d = inp("dv8", (128, npairs, 2, D), F8)  # [p, jp, c, d]: dv[jp*256+c*128+p, d]
    ones8_d = inp("ones8", (128, 2, 128), F8)
    wvT_d = inp("wvT", (D, D))            # Wv.T
    wg1T_d = inp("wg1T", (2 * D, D), BF16)  # Wg1.T
    wg2r_d = inp("wg2r", (128, D))        # Wg2 replicated over partitions
    bvr_d = inp("bvr", (128, D))
    bvg1_d = inp("bvg1", (1, D), BF16)    # bv @ Wg1a.T + bg1 (gate bias row)
    prevN_d = inp("prevN", (128, D))      # prev rows for this core's slice
    prevT_d = inp("prevT", (D, 128), BF16)  # same rows, transposed
    ident_d = inp("ident", (128, 128), BF16)
    out_d = nc.declare_dram_parameter("out", [128, D], F32, isOutput=True)

    rg = [list(range(NCORES))]
    DR = mybir.MatmulPerfMode.DoubleRow

    def emit_body(nc, tc, pools, rp):
        cp, sp, ep, xp, wgp, mmp, wvp, dp = pools

        def r32(ap):
            return ap.bitcast(mybir.dt.float32r)

        def cload(src_ap, shape, tg, dt=F32, eng=None, rounded=False):
            t = cp.tile(shape, dt, tag=tg, name=rp + tg)
            if rounded:
                (eng or nc.sync).dma_start(r32(t[:]), r32(src_ap))
            else:
                (eng or nc.sync).dma_start(t[:], src_ap)
            return t

        # --- loads ---------------------------------------------------------
        # SP(sync) queue: qk + datastore-K streams; ACT queue: datastore-V
        # (parallel DMA rings; ACT's issues land before the first exp).
        qkT8 = cp.tile([128, 4, SB], F8, tag="qkT8", name=rp + "qkT8")
        nc.sync.dma_start(qkT8[:, :, 0:512], qkT8_d[:, :, 0:512])
        dkT8 = cp.tile([128, 4, ns], F8, tag="dkT8", name=rp + "dkT8")
        nc.sync.dma_start(dkT8[:, :, 0:512], dkT8_d[:, :, 0:512])
        dv8 = cp.tile([128, npairs, 2, D], F8, tag="dv8", name=rp + "dv8")
        nc.sync.dma_start(dv8[:, 0:4, :, :], dv8_d[:, 0:4, :, :])
        nc.sync.dma_start(dkT8[:, :, 512:2048], dkT8_d[:, :, 512:2048])
        nc.sync.dma_start(dv8[:, 4:10, :, :], dv8_d[:, 4:10, :, :])
        nc.sync.dma_start(dkT8[:, :, 2048:ns], dkT8_d[:, :, 2048:ns])
        nc.sync.dma_start(dv8[:, 10:npairs, :, :], dv8_d[:, 10:npairs, :, :])
        nc.sync.dma_start(qkT8[:, :, 512:SB], qkT8_d[:, :, 512:SB])
        # remaining constants on the SP queue, behind the main streams
        ones8 = cload(ones8_d[:], [128, 2, 128], "ones8", dt=F8)
        wvT = [cload(wvT_d[k * 128:(k + 1) * 128, :], [128, D], f"wvT{k}",
                     rounded=True) for k in range(4)]
        wg2r = cload(wg2r_d[:], [128, D], "wg2r")
        bvr = cload(bvr_d[:], [128, D], "bvr")
        bvg1 = cload(bvg1_d[:], [1, D], "bvg1", dt=BF16)
        prevN = cload(prevN_d[:], [128, D], "prevN")
        prevT = [cload(prevT_d[k * 128:(k + 1) * 128, :], [128, 128], f"prevT{k}",
                       dt=BF16) for k in range(4)]
        ident = cload(ident_d[:], [128, 128], "identb", dt=BF16)
        ones = cp.tile([128, 1], F32, tag="ones", name=rp + "ones")
        nc.vector.memset(ones[:], 1.0)
        ebias = cp.tile([128, 1], F32, tag="ebias", name=rp + "ebias")
        nc.vector.memset(ebias[:], EBIAS)
        # [1,128] bf16 ones row: rank-1 bias accumulation in the gate matmul
        onesr = cp.tile([1, 128], BF16, tag="onesr", name=rp + "onesr")
        nc.vector.memset(onesr[:], 1.0)

        wvacc = [cp.tile([128, SB], F32, tag=f"wvacc{k}", name=rp + f"wvacc{k}")
                 for k in range(4)]
        S_sb = cp.tile([1, SB], F32, tag="S_sb", name=rp + "S_sb")
        eT = {}  # (h, jp) -> retained fp8 e tile

        cc_in = dp.tile([SB, 513], F32, tag="ccin", name=rp + "ccin")
        cc_out = dp.tile([SB // NCORES, 513], F32, tag="ccout", name=rp + "ccout")

        # --- main loop: logits -> exp -> AV, fp8 DoubleRow -----------------
        for h in range(2):
            sc = slice(h * 512, (h + 1) * 512)
            wv_ps = [wvp.tile([128, 512], F32, tag="wv", name=rp + f"wv{h}{k}")
                     for k in range(4)]
            for jp in range(npairs):
                pl2 = mmp.tile([128, 1024], F32, tag="mm", name=rp + f"pl{h}{jp}")
                for c in range(2):
                    j = jp * 2 + c
                    for kp in range(2):
                        nc.tensor.matmul(
                            pl2[:, c * 512:(c + 1) * 512],
                            dkT8[:, 2 * kp:2 * kp + 2, j * 128:(j + 1) * 128],
                            qkT8[:, 2 * kp:2 * kp + 2, sc],
                            start=(kp == 0), stop=(kp == 1), perf_mode=DR)
                et = ep.tile([128, 1024], F8, tag="e", name=rp + f"e{h}{jp}")
                nc.scalar.activation(et[:], pl2[:], AF.Exp,
                                     scale=1.0 / QS, bias=ebias[:])
                eT[(h, jp)] = et
                e3 = et[:].rearrange("p (c q) -> p c q", c=2)
                for k in range(4):
                    nc.tensor.matmul(
                        wv_ps[k][:], dv8[:, jp, :, k * 128:(k + 1) * 128], e3,
                        start=(jp == 0), stop=(jp == npairs - 1), perf_mode=DR)
            for k in range(4):
                nc.vector.tensor_copy(r32(wvacc[k][:, sc]), wv_ps[k][:])
            # sum-exp over this half's retained e tiles; for h=0 these PE ops
            # slot in at the head of h=1's loop, off the critical path.
            se_ps = wvp.tile([128, 512], F32, tag="wv", name=rp + f"se{h}")
            for jp in range(npairs):
                nc.tensor.matmul(
                    se_ps[:], ones8[:],
                    eT[(h, jp)][:].rearrange("p (c q) -> p c q", c=2),
                    start=(jp == 0), stop=(jp == npairs - 1), perf_mode=DR)
            nc.vector.tensor_copy(S_sb[0:1, sc], se_ps[0:1, :])

        # --- per-128-query-group: project partial AV through Wv.T, ship ----
        for g in range(8):
            pa = mmp.tile([128, 512], F32, tag="mm", name=rp + f"pa{g}")
            for k in range(4):
                nc.tensor.matmul(
                    pa[:], r32(wvacc[k][:, g * 128:(g + 1) * 128]),
                    r32(wvT[k][:]), start=(k == 0), stop=(k == 3))
            psc = wvp.tile([128, 1], F32, tag="wv", name=rp + f"psc{g}")
            nc.tensor.matmul(psc[:], S_sb[0:1, g * 128:(g + 1) * 128],
                             ones[0:1, 0:1], start=True, stop=True)
            ext = xp.tile([128, 513], F32, tag="ext", name=rp + f"ext{g}")
            # ACT is idle after the exp stream: do the wide PSUM->SBUF copies
            # there; DVE handles the 1-column sum-exp copies.
            nc.scalar.activation(ext[:, 0:512], pa[:], AF.Identity)
            nc.vector.tensor_copy(ext[:, 512:513], psc[:])
            nc.sync.dma_start(cc_in[g * 128:(g + 1) * 128, :], ext[:])
        # wg1T streamed now (bf16): the transfers ride out the RS wait.
        wg1T = []
        for k in range(8):
            t = wgp.tile([128, D], BF16, tag="wg", name=rp + f"wg1T{k}")
            nc.sync.dma_start(t[:], wg1T_d[k * 128:(k + 1) * 128, :])
            wg1T.append(t)
        # Gate algebra: with A := raw reduced AV (pre-recip), r := 1/S,
        #   h = relu(cat[A*r + bv, prev] @ Wg1.T + bg1)
        #     = r * relu(A @ Wg1a.T + S*(prev @ Wg1b.T + bv@Wg1a.T + bg1))
        # so the prev-side matmul + bias run DURING the RS, and r folds into
        # the sigmoid input scalar afterwards.
        phB = mmp.tile([128, D], F32, tag="mm", name=rp + "phB")
        for k in range(4):
            nc.tensor.matmul(phB[:], prevT[k][:], wg1T[4 + k][:],
                             start=(k == 0), stop=False)
        nc.tensor.matmul(phB[:], onesr[:], bvg1[:], start=False, stop=True)
        phBs = cp.tile([128, D], F32, tag="phBs", name=rp + "phBs")
        nc.vector.tensor_copy(phBs[:], phB[:])
        nc.gpsimd.collective_compute(
            "ReduceScatter", ALU.add, replica_groups=rg,
            ins=[cc_in.opt()], outs=[cc_out.opt()])

        # --- post-RS: this core's 128 query rows ---------------------------
        post = cp.tile([128, 513], F32, tag="post", name=rp + "post")
        nc.sync.dma_start(post[:], cc_out[:])
        recip = cp.tile([128, 1], F32, tag="recip", name=rp + "recip")
        nc.vector.reciprocal(recip[:], post[:, 512:513])
        # bf16 view of the raw AV columns for the transpose -> gate path
        postb = cp.tile([128, D], BF16, tag="postb", name=rp + "postb")
        nc.vector.tensor_copy(postb[:], post[:, 0:512])
        attn = sp.tile([128, D], F32, tag="scr", name=rp + "attn")
        nc.vector.scalar_tensor_tensor(
            attn[:], post[:, 0:512], recip[:], bvr[:],
            op0=ALU.mult, op1=ALU.add)
        dlt = sp.tile([128, D], F32, tag="scr", name=rp + "dlt")
        nc.vector.tensor_tensor(dlt[:], attn[:], prevN[:], op=ALU.subtract)

        aTall = cp.tile([128, D], BF16, tag="aTall", name=rp + "aTall")
        for k in range(4):
            pt = wvp.tile([128, 128], BF16, tag="wv", name=rp + f"pt{k}")
            nc.tensor.transpose(pt[:], postb[:, k * 128:(k + 1) * 128], ident[:])
            nc.vector.tensor_copy(aTall[:, k * 128:(k + 1) * 128], pt[:])
        phA = mmp.tile([128, D], F32, tag="mm", name=rp + "phA")
        for k in range(4):
            nc.tensor.matmul(phA[:], aTall[:, k * 128:(k + 1) * 128],
                             wg1T[k][:], start=(k == 0), stop=(k == 3))
        x = sp.tile([128, D], F32, tag="scr", name=rp + "x")
        nc.vector.scalar_tensor_tensor(
            x[:], phBs[:], post[:, 512:513], phA[:], op0=ALU.mult, op1=ALU.add)
        hrelu = sp.tile([128, D], F32, tag="scr", name=rp + "hrelu")
        nc.scalar.activation(hrelu[:], x[:], AF.Relu)

        tmp = sp.tile([128, D], F32, tag="scr", name=rp + "tmp")
        sigp = cp.tile([128, 1], F32, tag="sigp", name=rp + "sigp")
        nc.vector.scalar_tensor_tensor(
            tmp[:], hrelu[:], 1.0, wg2r[:],
            op0=ALU.mult, op1=ALU.mult, accum_out=sigp[:])
        sigin = cp.tile([128, 1], F32, tag="sigin", name=rp + "sigin")
        nc.vector.tensor_tensor(sigin[:], sigp[:], recip[:], op=ALU.mult)
        # sigma = 0.5 + 0.5*tanh(0.5*(x + bg2)); tanh shares the Exp table set
        tnh = cp.tile([128, 1], F32, tag="tnh", name=rp + "tnh")
        nc.scalar.activation(tnh[:], sigin[:], AF.Tanh,
                             scale=0.5, bias=0.5 * bg2f)
        sig1 = cp.tile([128, 1], F32, tag="sig1", name=rp + "sig1")
        nc.vector.tensor_scalar(sig1[:], tnh[:], 0.5, 0.5,
                                op0=ALU.mult, op1=ALU.add)
        res = sp.tile([128, D], F32, tag="scr", name=rp + "res")
        nc.vector.scalar_tensor_tensor(
            res[:], dlt[:], sig1[:], prevN[:], op0=ALU.mult, op1=ALU.add)
        nc.sync.dma_start(out_d[:], res[:])

    with tile.TileContext(nc) as tc:
        with (
            tc.tile_pool(name="const", bufs=1) as cp,
            tc.tile_pool(name="scratch", bufs=8) as sp,
            tc.tile_pool(name="ep", bufs=32) as ep,
            tc.tile_pool(name="xp", bufs=3) as xp,
            tc.tile_pool(name="wgp", bufs=8) as wgp,
            tc.tile_pool(name="mm", bufs=2, space="PSUM") as mmp,
            tc.tile_pool(name="wvp", bufs=4, space="PSUM") as wvp,
            tc.tile_pool(name="dram", bufs=1, space="DRAM") as dp,
        ):
            pools = (cp, sp, ep, xp, wgp, mmp, wvp, dp)
            for rep in range(reps):
                emit_body(nc, tc, pools, f"r{rep}_" if reps > 1 else "")

    nc.finalize()
    return nc


def make_in_maps(q, prev, Wq, bq, Wk, Wv, Wg1, Wg2, bg2, bv, bg1,
                 dstore_k, dstore_v, ns):
    """Host-side sharding + layout prep. Returns per-core input dicts."""
    alpha = (D ** -0.5) / TEMP
    f = np.float32
    qb = np.ascontiguousarray(q.transpose(1, 0, 2).reshape(SB, D), dtype=f)
    prevb = np.ascontiguousarray(prev.transpose(1, 0, 2).reshape(SB, D), dtype=f)
    wqk = (Wq.T.astype(np.float64) @ Wk.astype(np.float64) * alpha).astype(f)
    qkb = ((bq.astype(np.float64) @ Wk.astype(np.float64)) * alpha).astype(f)
    qk = qb @ wqk + qkb                      # [SB, D] projected scaled queries
    # [p, k, s] fp8 layout, pre-scaled by QS
    qkT8 = np.ascontiguousarray(
        (qk.T * QS).reshape(4, 128, SB).transpose(1, 0, 2)).astype(FP8_NP)
    wvT = np.ascontiguousarray(Wv.T.astype(f))
    wg1T = np.ascontiguousarray(Wg1.T.astype(BF16_NP))
    wg2r = np.ascontiguousarray(np.broadcast_to(Wg2.reshape(1, D), (128, D)), dtype=f)
    bvr = np.ascontiguousarray(np.broadcast_to(bv.reshape(1, D), (128, D)), dtype=f)
    bvg1 = np.ascontiguousarray(
        (bv.astype(np.float64) @ Wg1.T.astype(np.float64)[0:D]
         + bg1.astype(np.float64)).reshape(1, D)).astype(BF16_NP)
    ident = np.eye(128, dtype=BF16_NP)
    ones8 = np.ones((128, 2, 128), dtype=FP8_NP)
    npairs = ns // 256

    in_maps = []
    for c in range(NCORES):
        rows = slice(c * 128, (c + 1) * 128)
        prevN = np.ascontiguousarray(prevb[rows])
        prevT = np.ascontiguousarray(prevN.T.astype(BF16_NP))
        dk_s = dstore_k[c * ns:(c + 1) * ns, :].astype(f)
        dv_s = dstore_v[c * ns:(c + 1) * ns, :].astype(f)
        dkT8 = np.ascontiguousarray(
            dk_s.T.reshape(4, 128, ns).transpose(1, 0, 2)).astype(FP8_NP)
        dv8 = np.ascontiguousarray(
            dv_s.reshape(npairs, 2, 128, D).transpose(2, 0, 1, 3)).astype(FP8_NP)
        in_maps.append({
            "qkT8": qkT8, "dkT8": dkT8, "dv8": dv8, "ones8": ones8,
            "wvT": wvT, "wg1T": wg1T, "wg2r": wg2r, "bvr": bvr, "bvg1": bvg1,
            "prevN": prevN, "prevT": prevT, "ident": ident,
        })
    return in_maps


def assemble_output(core_outs):
    """[128,512] per core -> [SEQ, BATCH, D] full output."""
    res_bm = np.empty((SB, D), dtype=np.float32)
    for c in range(NCORES):
        res_bm[c * 128:(c + 1) * 128] = core_outs[c]
    return np.ascontiguousarray(
        res_bm.reshape(BATCH, SEQ, D).transpose(1, 0, 2))


def kernel(q, prev_layer_output, Wq, bq, Wk, bk, Wv, bv, Wg1, bg1, Wg2, bg2,
           dstore_k, dstore_v):
    # bk shifts every logit in a row by a constant -> softmax-invariant; unused.
    ns = NTOT // NCORES
    bg2f = float(np.asarray(bg2).reshape(-1)[0])
    key = (ns, bg2f, 1)
    if key not in _PROGRAM_CACHE:
        _PROGRAM_CACHE[key] = build_program(ns, bg2f)
    nc = _PROGRAM_CACHE[key]
    in_maps = make_in_maps(q, prev_layer_output, Wq, bq, Wk, Wv, Wg1, Wg2, bg2,
                           bv, bg1, dstore_k, dstore_v, ns)
    res = run_bass_kernel_spmd(nc, in_maps, list(range(NCORES)))
    return assemble_output([res.results[c]["out"] for c in range(NCORES)])


# revision 42
# speedup vs baseline: 2.6153x; 1.3008x over previous
"""Trainium2 Bass kernel for nn_DatastoreReaderLayer (retrieval kNN attention).

Strategy (8 NeuronCores, datastore sharded over N):
  - Each core owns an N/8 = 4096-row shard of the datastore.
  - K/V weight projections are algebraically absorbed:
      logits = qk @ dstore_k.T   where qk := alpha * (qb @ Wq.T + bq) @ Wk
      attn   = (softmax @ dstore_v) @ Wv.T
    qk (a [1024, 512] projection, 0.25% of total FLOPs) is folded on host;
    the O(N) retrieval work runs on device.
  - fp8 (e4m3) main loop with DoubleRow matmuls (2 k-subtiles per pass,
    0.5 PE cycles/row): logits and AV both contract 256-deep per matmul.
    qk is scaled by QS=64 into fp8 range (undone by the exp's scale);
    exp output is biased by ln(1/2) so e stays within e4m3's +/-240
    (the 1/2 is a global softmax constant and cancels).
  - Softmax without max-subtraction (logits are in [-5.7, 5.5] for this
    distribution; exp fits fp8 with the 1/2 bias).
  - Per-core partial sum-exp (from retained fp8 e tiles, contracted with a
    ones vector per half) and partial unnormalized AV are combined across
    cores with ONE bf16 ReduceScatter over all 1024 query rows ([1024, 513]
    payload: 512 attn columns + 1 sum-exp column).
  - Each core finishes the gate MLP (bf16 weights) for its own 128 query
    rows. Using relu(r*x) = r*relu(x) for r = 1/sum-exp > 0, the prev-side
    gate matmul + fused bias row run during the ReduceScatter; only the
    attn-side matmul and a short fused elementwise chain remain after it.
  - bk is provably a softmax no-op; bv/bg1/bg2 are applied exactly on device.
"""

import sys

for _p in ("/opt/trn_rl_repo", "/root/.axon_site/_ro/trn_rl_repo"):
    if _p not in sys.path:
        sys.path.append(_p)

import numpy as np
import ml_dtypes

import concourse.tile as tile
from concourse import bacc, mybir
from concourse.bass_utils import run_bass_kernel_spmd

SEQ, BATCH, D, NTOT = 256, 4, 512, 32768
TEMP = 0.5
NCORES = 8
SB = SEQ * BATCH  # 1024 query rows, b-major (row r = b*SEQ + s)
F32 = mybir.dt.float32
BF16 = mybir.dt.bfloat16
F8 = mybir.dt.float8e4
AF = mybir.ActivationFunctionType
ALU = mybir.AluOpType
FP8_NP = ml_dtypes.float8_e4m3
BF16_NP = ml_dtypes.bfloat16

QS = 64.0            # qk pre-scale into fp8 normal range
EBIAS = -0.6931472   # exp bias: e' = 0.5 * e^l keeps e' <= 120 < 240 (fp8 max)

_PROGRAM_CACHE: dict = {}


def build_program(ns: int, bg2f: float, reps: int = 1):
    """One SPMD program; per-core data differences come via in_maps.

    reps > 1 statically repeats the whole computation (for wall-clock-delta
    timing of the kernel proper); the output is written identically each rep.
    """
    nchunks = ns // 128          # 32
    npairs = nchunks // 2        # 16 chunk-pairs (DoubleRow contracts 2)
    nc = bacc.Bacc(None, target_bir_lowering=False, debug=False, num_devices=NCORES)

    def inp(nm, shp, dt=F32):
        return nc.declare_dram_parameter(nm, list(shp), dt, isOutput=False)

    qkT8_d = inp("qkT8", (128, 4, SB), F8)       # [p, k, s]: qk[s, k*128+p]*QS
    dkT8_d = inp("dkT8", (128, 4, ns), F8)       # [p, k, n]: dk[n, k*128+p]
    dv8_d = inp("dv8", (128, npairs, 2, D), F8)  # [p, jp, c, d]: dv[jp*256+c*128+p, d]
    ones8_d = inp("ones8", (128, 2, 128), F8)
    wvT_d = inp("wvT", (D, D))            # Wv.T
    wg1T_d = inp("wg1T", (2 * D, D), BF16)  # Wg1.T
    wg2r_d = inp("wg2r", (128, D))        # Wg2 replicated over partitions
    bvr_d = inp("bvr", (128, D))
    bvg1_d = inp("bvg1", (1, D), BF16)    # bv @ Wg1a.T + bg1 (gate bias row)
    prevN_d = inp("prevN", (128, D))      # prev rows for this core's slice
    prevT_d = inp("prevT", (D, 128), BF16)  # same rows, transposed
    ident_d = inp("ident", (128, 128), BF16)
    out_d = nc.declare_dram_parameter("out", [128, D], F32, isOutput=True)

    rg = [list(range(NCORES))]
    DR = mybir.MatmulPerfMode.DoubleRow

    def emit_body(nc, tc, pools, rp):
        cp, sp, ep, xp, wgp, mmp, wvp, dp = pools

        def r32(ap):
            return ap.bitcast(mybir.dt.float32r)

        def cload(src_ap, shape, tg, dt=F32, eng=None, rounded=False):
            t = cp.tile(shape, dt, tag=tg, name=rp + tg)
            if rounded:
                (eng or nc.sync).dma_start(r32(t[:]), r32(src_ap))
            else:
                (eng or nc.sync).dma_start(t[:], src_ap)
            return t

        # --- loads ---------------------------------------------------------
        # SP(sync) queue: qk + datastore-K streams; ACT queue: datastore-V
        # (parallel DMA rings; ACT's issues land before the first exp).
        qkT8 = cp.tile([128, 4, SB], F8, tag="qkT8", name=rp + "qkT8")
        nc.sync.dma_start(qkT8[:, :, 0:512], qkT8_d[:, :, 0:512])
        dkT8 = cp.tile([128, 4, ns], F8, tag="dkT8", name=rp + "dkT8")
        nc.sync.dma_start(dkT8[:, :, 0:512], dkT8_d[:, :, 0:512])
        dv8 = cp.tile([128, npairs, 2, D], F8, tag="dv8", name=rp + "dv8")
        nc.sync.dma_start(dv8[:, 0:4, :, :], dv8_d[:, 0:4, :, :])
        nc.sync.dma_start(dkT8[:, :, 512:2048], dkT8_d[:, :, 512:2048])
        nc.sync.dma_start(dv8[:, 4:10, :, :], dv8_d[:, 4:10, :, :])
        nc.sync.dma_start(dkT8[:, :, 2048:ns], dkT8_d[:, :, 2048:ns])
        nc.sync.dma_start(dv8[:, 10:npairs, :, :], dv8_d[:, 10:npairs, :, :])
        nc.sync.dma_start(qkT8[:, :, 512:SB], qkT8_d[:, :, 512:SB])
        # remaining constants on the SP queue, behind the main streams
        ones8 = cload(ones8_d[:], [128, 2, 128], "ones8", dt=F8)
        wvT = [cload(wvT_d[k * 128:(k + 1) * 128, :], [128, D], f"wvT{k}",
                     rounded=True) for k in range(4)]
        wg2r = cload(wg2r_d[:], [128, D], "wg2r")
        bvr = cload(bvr_d[:], [128, D], "bvr")
        bvg1 = cload(bvg1_d[:], [1, D], "bvg1", dt=BF16)
        prevN = cload(prevN_d[:], [128, D], "prevN")
        prevT = [cload(prevT_d[k * 128:(k + 1) * 128, :], [128, 128], f"prevT{k}",
                       dt=BF16) for k in range(4)]
        ident = cload(ident_d[:], [128, 128], "identb", dt=BF16)
        ones = cp.tile([128, 1], F32, tag="ones", name=rp + "ones")
        nc.vector.memset(ones[:], 1.0)
        ebias = cp.tile([128, 1], F32, tag="ebias", name=rp + "ebias")
        nc.vector.memset(ebias[:], EBIAS)
        # [1,128] bf16 ones row: rank-1 bias accumulation in the gate matmul
        onesr = cp.tile([1, 128], BF16, tag="onesr", name=rp + "onesr")
        nc.vector.memset(onesr[:], 1.0)

        wvacc = [cp.tile([128, SB], F32, tag=f"wvacc{k}", name=rp + f"wvacc{k}")
                 for k in range(4)]
        S_sb = cp.tile([1, SB], F32, tag="S_sb", name=rp + "S_sb")
        eT = {}  # (h, jp) -> retained fp8 e tile

        cc_in = dp.tile([SB, 513], BF16, tag="ccin", name=rp + "ccin")
        cc_out = dp.tile([SB // NCORES, 513], BF16, tag="ccout", name=rp + "ccout")

        # --- main loop: logits -> exp -> AV, fp8 DoubleRow -----------------
        for h in range(2):
            sc = slice(h * 512, (h + 1) * 512)
            wv_ps = [wvp.tile([128, 512], F32, tag="wv", name=rp + f"wv{h}{k}")
                     for k in range(4)]
            for jp in range(npairs):
                pl2 = mmp.tile([128, 1024], F32, tag="mm", name=rp + f"pl{h}{jp}")
                for c in range(2):
                    j = jp * 2 + c
                    for kp in range(2):
                        nc.tensor.matmul(
                            pl2[:, c * 512:(c + 1) * 512],
                            dkT8[:, 2 * kp:2 * kp + 2, j * 128:(j + 1) * 128],
                            qkT8[:, 2 * kp:2 * kp + 2, sc],
                            start=(kp == 0), stop=(kp == 1), perf_mode=DR)
                et = ep.tile([128, 1024], F8, tag="e", name=rp + f"e{h}{jp}")
                nc.scalar.activation(et[:], pl2[:], AF.Exp,
                                     scale=1.0 / QS, bias=ebias[:])
                eT[(h, jp)] = et
                e3 = et[:].rearrange("p (c q) -> p c q", c=2)
                for k in range(4):
                    nc.tensor.matmul(
                        wv_ps[k][:], dv8[:, jp, :, k * 128:(k + 1) * 128], e3,
                        start=(jp == 0), stop=(jp == npairs - 1), perf_mode=DR)
            for k in range(4):
                nc.vector.tensor_copy(r32(wvacc[k][:, sc]), wv_ps[k][:])
            # sum-exp over this half's retained e tiles; for h=0 these PE ops
            # slot in at the head of h=1's loop, off the critical path.
            se_ps = wvp.tile([128, 512], F32, tag="wv", name=rp + f"se{h}")
            for jp in range(npairs):
                nc.tensor.matmul(
                    se_ps[:], ones8[:],
                    eT[(h, jp)][:].rearrange("p (c q) -> p c q", c=2),
                    start=(jp == 0), stop=(jp == npairs - 1), perf_mode=DR)
            nc.vector.tensor_copy(S_sb[0:1, sc], se_ps[0:1, :])

        # --- per-128-query-group: project partial AV through Wv.T, ship ----
        for g in range(8):
            pa = mmp.tile([128, 512], F32, tag="mm", name=rp + f"pa{g}")
            for k in range(4):
                nc.tensor.matmul(
                    pa[:], r32(wvacc[k][:, g * 128:(g + 1) * 128]),
                    r32(wvT[k][:]), start=(k == 0), stop=(k == 3))
            psc = wvp.tile([128, 1], F32, tag="wv", name=rp + f"psc{g}")
            nc.tensor.matmul(psc[:], S_sb[0:1, g * 128:(g + 1) * 128],
                             ones[0:1, 0:1], start=True, stop=True)
            ext = xp.tile([128, 513], BF16, tag="ext", name=rp + f"ext{g}")
            # ACT is idle after the exp stream: do the wide PSUM->SBUF copies
            # there; DVE handles the 1-column sum-exp copies.
            nc.scalar.activation(ext[:, 0:512], pa[:], AF.Identity)
            nc.vector.tensor_copy(ext[:, 512:513], psc[:])
            nc.sync.dma_start(cc_in[g * 128:(g + 1) * 128, :], ext[:])
        # wg1T streamed now (bf16): the transfers ride out the RS wait.
        wg1T = []
        for k in range(8):
            t = wgp.tile([128, D], BF16, tag="wg", name=rp + f"wg1T{k}")
            nc.sync.dma_start(t[:], wg1T_d[k * 128:(k + 1) * 128, :])
            wg1T.append(t)
        # Gate algebra: with A := raw reduced AV (pre-recip), r := 1/S,
        #   h = relu(cat[A*r + bv, prev] @ Wg1.T + bg1)
        #     = r * relu(A @ Wg1a.T + S*(prev @ Wg1b.T + bv@Wg1a.T + bg1))
        # so the prev-side matmul + bias run DURING the RS, and r folds into
        # the sigmoid input scalar afterwards.
        phB = mmp.tile([128, D], F32, tag="mm", name=rp + "phB")
        for k in range(4):
            nc.tensor.matmul(phB[:], prevT[k][:], wg1T[4 + k][:],
                             start=(k == 0), stop=False)
        nc.tensor.matmul(phB[:], onesr[:], bvg1[:], start=False, stop=True)
        phBs = cp.tile([128, D], F32, tag="phBs", name=rp + "phBs")
        nc.vector.tensor_copy(phBs[:], phB[:])
        nc.gpsimd.collective_compute(
            "ReduceScatter", ALU.add, replica_groups=rg,
            ins=[cc_in.opt()], outs=[cc_out.opt()])

        # --- post-RS: this core's 128 query rows ---------------------------
        post = cp.tile([128, 513], BF16, tag="post", name=rp + "post")
        nc.sync.dma_start(post[:], cc_out[:])
        recip = cp.tile([128, 1], F32, tag="recip", name=rp + "recip")
        nc.vector.reciprocal(recip[:], post[:, 512:513])
        attn = sp.tile([128, D], F32, tag="scr", name=rp + "attn")
        nc.vector.scalar_tensor_tensor(
            attn[:], post[:, 0:512], recip[:], bvr[:],
            op0=ALU.mult, op1=ALU.add)
        dlt = sp.tile([128, D], F32, tag="scr", name=rp + "dlt")
        nc.vector.tensor_tensor(dlt[:], attn[:], prevN[:], op=ALU.subtract)

        aTall = cp.tile([128, D], BF16, tag="aTall", name=rp + "aTall")
        for k in range(4):
            pt = wvp.tile([128, 128], BF16, tag="wv", name=rp + f"pt{k}")
            nc.tensor.transpose(pt[:], post[:, k * 128:(k + 1) * 128], ident[:])
            nc.vector.tensor_copy(aTall[:, k * 128:(k + 1) * 128], pt[:])
        phA = mmp.tile([128, D], F32, tag="mm", name=rp + "phA")
        for k in range(4):
            nc.tensor.matmul(phA[:], aTall[:, k * 128:(k + 1) * 128],
                             wg1T[k][:], start=(k == 0), stop=(k == 3))
        x = sp.tile([128, D], F32, tag="scr", name=rp + "x")
        nc.vector.scalar_tensor_tensor(
            x[:], phBs[:], post[:, 512:513], phA[:], op0=ALU.mult, op1=ALU.add)
        hrelu = sp.tile([128, D], F32, tag="scr", name=rp + "hrelu")
        nc.scalar.activation(hrelu[:], x[:], AF.Relu)

        tmp = sp.tile([128, D], F32, tag="scr", name=rp + "tmp")
        sigp = cp.tile([128, 1], F32, tag="sigp", name=rp + "sigp")
        nc.vector.scalar_tensor_tensor(
            tmp[:], hrelu[:], 1.0, wg2r[:],
            op0=ALU.mult, op1=ALU.mult, accum_out=sigp[:])
        sigin = cp.tile([128, 1], F32, tag="sigin", name=rp + "sigin")
        nc.vector.tensor_tensor(sigin[:], sigp[:], recip[:], op=ALU.mult)
        # sigma = 0.5 + 0.5*tanh(0.5*(x + bg2)); tanh shares the Exp table set
        tnh = cp.tile([128, 1], F32, tag="tnh", name=rp + "tnh")
        nc.scalar.activation(tnh[:], sigin[:], AF.Tanh,
                             scale=0.5, bias=0.5 * bg2f)
        sig1 = cp.tile([128, 1], F32, tag="sig1", name=rp + "sig1")
        nc.vector.tensor_scalar(sig1[:], tnh[:], 0.5, 0.5,
                                op0=ALU.mult, op1=ALU.add)
        res = sp.tile([128, D], F32, tag="scr", name=rp + "res")
        nc.vector.scalar_tensor_tensor(
            res[:], dlt[:], sig1[:], prevN[:], op0=ALU.mult, op1=ALU.add)
        nc.sync.dma_start(out_d[:], res[:])

    with tile.TileContext(nc) as tc:
        with (
            tc.tile_pool(name="const", bufs=1) as cp,
            tc.tile_pool(name="scratch", bufs=8) as sp,
            tc.tile_pool(name="ep", bufs=32) as ep,
            tc.tile_pool(name="xp", bufs=3) as xp,
            tc.tile_pool(name="wgp", bufs=8) as wgp,
            tc.tile_pool(name="mm", bufs=2, space="PSUM") as mmp,
            tc.tile_pool(name="wvp", bufs=4, space="PSUM") as wvp,
            tc.tile_pool(name="dram", bufs=1, space="DRAM") as dp,
        ):
            pools = (cp, sp, ep, xp, wgp, mmp, wvp, dp)
            for rep in range(reps):
                emit_body(nc, tc, pools, f"r{rep}_" if reps > 1 else "")

    nc.finalize()
    return nc


def make_in_maps(q, prev, Wq, bq, Wk, Wv, Wg1, Wg2, bg2, bv, bg1,
                 dstore_k, dstore_v, ns):
    """Host-side sharding + layout prep. Returns per-core input dicts."""
    alpha = (D ** -0.5) / TEMP
    f = np.float32
    qb = np.ascontiguousarray(q.transpose(1, 0, 2).reshape(SB, D), dtype=f)
    prevb = np.ascontiguousarray(prev.transpose(1, 0, 2).reshape(SB, D), dtype=f)
    wqk = (Wq.T.astype(np.float64) @ Wk.astype(np.float64) * alpha).astype(f)
    qkb = ((bq.astype(np.float64) @ Wk.astype(np.float64)) * alpha).astype(f)
    qk = qb @ wqk + qkb                      # [SB, D] projected scaled queries
    # [p, k, s] fp8 layout, pre-scaled by QS
    qkT8 = np.ascontiguousarray(
        (qk.T * QS).reshape(4, 128, SB).transpose(1, 0, 2)).astype(FP8_NP)
    wvT = np.ascontiguousarray(Wv.T.astype(f))
    wg1T = np.ascontiguousarray(Wg1.T.astype(BF16_NP))
    wg2r = np.ascontiguousarray(np.broadcast_to(Wg2.reshape(1, D), (128, D)), dtype=f)
    bvr = np.ascontiguousarray(np.broadcast_to(bv.reshape(1, D), (128, D)), dtype=f)
    bvg1 = np.ascontiguousarray(
        (bv.astype(np.float64) @ Wg1.T.astype(np.float64)[0:D]
         + bg1.astype(np.float64)).reshape(1, D)).astype(BF16_NP)
    ident = np.eye(128, dtype=BF16_NP)
    ones8 = np.ones((128, 2, 128), dtype=FP8_NP)
    npairs = ns // 256

    in_maps = []
    for c in range(NCORES):
        rows = slice(c * 128, (c + 1) * 128)
        prevN = np.ascontiguousarray(prevb[rows])
        prevT = np.ascontiguousarray(prevN.T.astype(BF16_NP))
        dk_s = dstore_k[c * ns:(c + 1) * ns, :].astype(f)
        dv_s = dstore_v[c * ns:(c + 1) * ns, :].astype(f)
        dkT8 = np.ascontiguousarray(
            dk_s.T.reshape(4, 128, ns).transpose(1, 0, 2)).astype(FP8_NP)
        dv8 = np.ascontiguousarray(
            dv_s.reshape(npairs, 2, 128, D).transpose(2, 0, 1, 3)).astype(FP8_NP)
        in_maps.append({
            "qkT8": qkT8, "dkT8": dkT8, "dv8": dv8, "ones8": ones8,
            "wvT": wvT, "wg1T": wg1T, "wg2r": wg2r, "bvr": bvr, "bvg1": bvg1,
            "prevN": prevN, "prevT": prevT, "ident": ident,
        })
    return in_maps


def assemble_output(core_outs):
    """[128,512] per core -> [SEQ, BATCH, D] full output."""
    res_bm = np.empty((SB, D), dtype=np.float32)
    for c in range(NCORES):
        res_bm[c * 128:(c + 1) * 128] = core_outs[c]
    return np.ascontiguousarray(
        res_bm.reshape(BATCH, SEQ, D).transpose(1, 0, 2))


def kernel(q, prev_layer_output, Wq, bq, Wk, bk, Wv, bv, Wg1, bg1, Wg2, bg2,
           dstore_k, dstore_v):
    # bk shifts every logit in a row by a constant -> softmax-invariant; unused.
    ns = NTOT // NCORES
    bg2f = float(np.asarray(bg2).reshape(-1)[0])
    key = (ns, bg2f, 1)
    if key not in _PROGRAM_CACHE:
        _PROGRAM_CACHE[key] = build_program(ns, bg2f)
    nc = _PROGRAM_CACHE[key]
    in_maps = make_in_maps(q, prev_layer_output, Wq, bq, Wk, Wv, Wg1, Wg2, bg2,
                           bv, bg1, dstore_k, dstore_v, ns)
    res = run_bass_kernel_spmd(nc, in_maps, list(range(NCORES)))
    return assemble_output([res.results[c]["out"] for c in range(NCORES)])
